# revision 5
# baseline (speedup 1.0000x reference)
"""AdditiveAttention Bass kernel for 8 Trainium2 NeuronCores.

Math (reference):
    q = queries @ W_q            [B,Q,H]
    k = keys @ W_k               [B,K,H]
    scores[b,q,k] = sum_h w_v[h] * tanh(q[b,q,h] + k[b,k,h])
    attn = softmax(mask(scores)) over K
    out = attn @ values          [B,Q,D]

Key observations exploited here:
  * Masked positions (k >= valid_len[b]) contribute exactly zero to the
    softmax (exp(-1e6 - max) underflows to 0 in fp32), so they can be
    skipped entirely.  valid_lens is host-visible inside kernel(), so the
    work list is built at (host) compile time.
  * |scores| <= ||w_v||_1 ~= 13, so softmax without max-subtraction is
    numerically safe in fp32; partial sums (o = sum exp(s)*v, z = sum
    exp(s)) are therefore linear and can be summed across chunks on host.
  * Work is packed into uniform (batch, key-chunk-of-128) tasks spread
    round-robin over the 8 cores -> a single SPMD program, perfectly
    load-balanced regardless of the valid_lens distribution.

Per-task device pipeline (C = 128 keys/task):
    PE : q_proj/k_proj projections (H on partitions)
    DVE: qk[h, q, c] = k_proj[h, c] + q_proj[h, q]   (per-partition scalar add)
    ACT: feat = tanh(qk) -> bf16, large free-dim instructions
    PE : scoresT[c, q] = feat[h,(q),c].T @ w_v       (feat as stationary)
    ACT: p = exp(scoresT)                            [c, q]
    PE : o[d, q] = V[c,d].T @ p ; z[q] = mask.T @ p  (mask via zeroed V rows)
Host combines per-batch partials: out[b] = (sum_t o_t) / (sum_t z_t).
"""

import math
from contextlib import ExitStack

import numpy as np
import ml_dtypes

import concourse.bass as bass
import concourse.mybir as mybir
import concourse.tile as tile
from concourse import bacc, bass_utils

F32 = mybir.dt.float32
BF16 = mybir.dt.bfloat16

B, Q, K, D, H = 16, 64, 1024, 256, 256
C = 128          # keys per task
GQ = 8           # queries per tanh group
N_CORES = 8
DC = D // 128    # d chunks (2)
HC = H // 128    # h chunks (2)


def emit_kernel(tc, aps, n_tasks):
    """Emit the per-core SPMD program for n_tasks uniform tasks."""
    nc = tc.nc
    ctx = tc.ctx  # ExitStack owned by caller

    keysT = aps["keysT"]        # [T, 128, DC, C] f32   (dp, dc, c)
    queriesT = aps["queriesT"]  # [T, 128, DC, Q] f32
    vals = aps["vals"]          # [T, 128, D] f32       (c, d) lhsT for o-matmul
    maskv = aps["maskv"]        # [T, 128, 1] f32
    Wq = aps["Wq"]              # [128, DC, H] f32      (dp, dc, h)
    Wk = aps["Wk"]              # [128, DC, H] f32
    wv = aps["wv"]              # [128, HC] bf16
    o_out = aps["o_out"]        # [T, 128, DC, Q] f32   (dp, dc, q)
    s_out = aps["s_out"]        # [T, 1, Q] f32

    const_pool = ctx.enter_context(tc.tile_pool(name="const", bufs=1))
    in_pool = ctx.enter_context(tc.tile_pool(name="inp", bufs=2))
    proj_pool = ctx.enter_context(tc.tile_pool(name="proj", bufs=2))
    qk_pool = ctx.enter_context(tc.tile_pool(name="qk", bufs=3))
    feat_pool = ctx.enter_context(tc.tile_pool(name="feat", bufs=4))
    p_pool = ctx.enter_context(tc.tile_pool(name="p", bufs=2))
    out_pool = ctx.enter_context(tc.tile_pool(name="outp", bufs=2))
    ps_proj = ctx.enter_context(tc.tile_pool(name="psproj", bufs=2, space="PSUM"))
    ps_sc = ctx.enter_context(tc.tile_pool(name="pssc", bufs=2, space="PSUM"))
    ps_o = ctx.enter_context(tc.tile_pool(name="pso", bufs=2, space="PSUM"))

    # Resident constants.
    Wq_sb = const_pool.tile([128, DC, H], F32, tag="wq")
    Wk_sb = const_pool.tile([128, DC, H], F32, tag="wk")
    wv_sb = const_pool.tile([128, HC], BF16, tag="wv")
    nc.sync.dma_start(Wq_sb[:], Wq[:])
    nc.sync.dma_start(Wk_sb[:], Wk[:])
    nc.sync.dma_start(wv_sb[:], wv[:])

    for t in range(n_tasks):
        # ---- input DMA ----
        k_sb = in_pool.tile([128, DC, C], F32, tag="k")
        qT_sb = in_pool.tile([128, DC, Q], F32, tag="q")
        v_sb = in_pool.tile([128, D], F32, tag="v")
        m_sb = in_pool.tile([128, 1], F32, tag="m")
        nc.sync.dma_start(k_sb[:], keysT[t])
        nc.sync.dma_start(qT_sb[:], queriesT[t])
        nc.sync.dma_start(v_sb[:], vals[t])
        nc.sync.dma_start(m_sb[:], maskv[t])

        # ---- projections (PE): proj_ps[:, hh, 0:Q] = q_proj, [, Q:Q+C] = k_proj
        proj_ps = ps_proj.tile([128, HC, Q + C], F32, tag="proj")
        for hh in range(HC):
            for dc in range(DC):
                nc.tensor.matmul(
                    proj_ps[:, hh, 0:Q],
                    lhsT=Wq_sb[:, dc, hh * 128:(hh + 1) * 128],
                    rhs=qT_sb[:, dc, :],
                    start=(dc == 0), stop=(dc == DC - 1),
                )
            for dc in range(DC):
                nc.tensor.matmul(
                    proj_ps[:, hh, Q:Q + C],
                    lhsT=Wk_sb[:, dc, hh * 128:(hh + 1) * 128],
                    rhs=k_sb[:, dc, :],
                    start=(dc == 0), stop=(dc == DC - 1),
                )
        qp_sb = proj_pool.tile([128, HC, Q], F32, tag="qp")
        kp_sb = proj_pool.tile([128, HC, C], F32, tag="kp")
        nc.scalar.copy(qp_sb[:], proj_ps[:, :, 0:Q])
        nc.vector.tensor_copy(kp_sb[:], proj_ps[:, :, Q:Q + C])

        # ---- qk broadcast-add (DVE) + tanh (ACT), grouped by GQ queries ----
        feats = []
        for g in range(Q // GQ):
            qk = qk_pool.tile([128, GQ, HC, C], F32, tag="qk")
            for i in range(GQ):
                qq = g * GQ + i
                for hh in range(HC):
                    nc.vector.tensor_scalar_add(
                        qk[:, i, hh, :],
                        kp_sb[:, hh, :],
                        qp_sb[:, hh, qq:qq + 1],
                    )
            feat = feat_pool.tile([128, GQ, HC, C], BF16, tag="feat")
            nc.scalar.activation(feat[:], qk[:], mybir.ActivationFunctionType.Tanh)
            feats.append(feat)

        # ---- scoresT[c, q] (PE): feat as stationary, w_v streaming ----
        sc_ps = ps_sc.tile([128, Q + Q], F32, tag="sc")  # [:, :Q] scoresT, [0:1, Q:] z
        for qq in range(Q):
            g, i = divmod(qq, GQ)
            for hh in range(HC):
                nc.tensor.matmul(
                    sc_ps[:, qq:qq + 1],
                    lhsT=feats[g][:, i, hh, :],
                    rhs=wv_sb[:, hh:hh + 1],
                    start=(hh == 0), stop=(hh == HC - 1),
                )

        # ---- exp (ACT) ----
        p_sb = p_pool.tile([128, Q], F32, tag="p")
        nc.scalar.activation(p_sb[:], sc_ps[:, 0:Q], mybir.ActivationFunctionType.Exp)

        # ---- o = V.T @ p, z = mask.T @ p (PE) ----
        o_ps = ps_o.tile([128, DC, Q], F32, tag="o")
        for dc in range(DC):
            nc.tensor.matmul(
                o_ps[:, dc, :],
                lhsT=v_sb[:, dc * 128:(dc + 1) * 128],
                rhs=p_sb[:],
                start=True, stop=True,
            )
        nc.tensor.matmul(
            sc_ps[0:1, Q:Q + Q],
            lhsT=m_sb[:],
            rhs=p_sb[:],
            start=True, stop=True,
        )

        # ---- evacuate + output DMA ----
        o_sb = out_pool.tile([128, DC, Q], F32, tag="osb")
        s_sb = out_pool.tile([1, Q], F32, tag="ssb")
        nc.scalar.copy(o_sb[:], o_ps[:])
        nc.scalar.copy(s_sb[:], sc_ps[0:1, Q:Q + Q])
        nc.sync.dma_start(o_out[t], o_sb[:])
        nc.sync.dma_start(s_out[t], s_sb[:])


_NC_CACHE = {}


def build_nc(n_tasks):
    if n_tasks in _NC_CACHE:
        return _NC_CACHE[n_tasks]
    nc = bacc.Bacc("TRN2", target_bir_lowering=False, debug=False)
    aps = {
        "keysT": nc.dram_tensor("keysT", [n_tasks, 128, DC, C], F32,
                                kind="ExternalInput").ap(),
        "queriesT": nc.dram_tensor("queriesT", [n_tasks, 128, DC, Q], F32,
                                   kind="ExternalInput").ap(),
        "vals": nc.dram_tensor("vals", [n_tasks, 128, D], F32,
                               kind="ExternalInput").ap(),
        "maskv": nc.dram_tensor("maskv", [n_tasks, 128, 1], F32,
                                kind="ExternalInput").ap(),
        "Wq": nc.dram_tensor("Wq", [128, DC, H], F32, kind="ExternalInput").ap(),
        "Wk": nc.dram_tensor("Wk", [128, DC, H], F32, kind="ExternalInput").ap(),
        "wv": nc.dram_tensor("wv", [128, HC], BF16, kind="ExternalInput").ap(),
        "o_out": nc.dram_tensor("o_out", [n_tasks, 128, DC, Q], F32,
                                kind="ExternalOutput").ap(),
        "s_out": nc.dram_tensor("s_out", [n_tasks, 1, Q], F32,
                                kind="ExternalOutput").ap(),
    }
    with tile.TileContext(nc) as tc:
        with ExitStack() as stack:
            tc.ctx = stack
            emit_kernel(tc, aps, n_tasks)
    nc.compile()
    _NC_CACHE[n_tasks] = (nc, aps)
    return nc, aps


def make_task_list(valid_lens):
    """Uniform (b, c0) tasks; None = dummy task. Returns (tasks, n_tasks)."""
    chunks = []
    for b in range(B):
        v = int(valid_lens[b])
        for c0 in range(0, v, C):
            chunks.append((b, c0))
    n_tasks = math.ceil(len(chunks) / N_CORES)
    chunks += [None] * (n_tasks * N_CORES - len(chunks))
    per_core = [chunks[i * n_tasks:(i + 1) * n_tasks] for i in range(N_CORES)]
    return per_core, n_tasks


def pack_inputs(queries, keys, values, valid_lens, W_q, W_k, w_v, per_core, n_tasks):
    """Build the per-core input maps (all little host-side layout work)."""
    Wq_arr = np.ascontiguousarray(
        W_q.reshape(DC, 128, H).transpose(1, 0, 2))       # [128, DC, H]
    Wk_arr = np.ascontiguousarray(
        W_k.reshape(DC, 128, H).transpose(1, 0, 2))
    wv_arr = np.ascontiguousarray(
        w_v.reshape(HC, 128).T.astype(ml_dtypes.bfloat16))  # [128, HC]

    in_maps = []
    for core in range(N_CORES):
        keysT = np.zeros((n_tasks, 128, DC, C), np.float32)
        queriesT = np.zeros((n_tasks, 128, DC, Q), np.float32)
        vals = np.zeros((n_tasks, 128, D), np.float32)
        maskv = np.zeros((n_tasks, 128, 1), np.float32)
        for t, task in enumerate(per_core[core]):
            if task is None:
                continue
            b, c0 = task
            v = int(valid_lens[b])
            n = min(C, v - c0)
            # keys[b, c0:c0+n, :] -> [D, n] -> [128dp, DC, n]
            kT = keys[b, c0:c0 + n, :].T.reshape(DC, 128, n)
            keysT[t, :, :, :n] = kT.transpose(1, 0, 2)
            qT = queries[b].T.reshape(DC, 128, Q)
            queriesT[t] = qT.transpose(1, 0, 2)
            vals[t, :n, :] = values[b, c0:c0 + n, :]
            maskv[t, :n, 0] = 1.0
        in_maps.append({
            "keysT": keysT, "queriesT": queriesT, "vals": vals, "maskv": maskv,
            "Wq": Wq_arr, "Wk": Wk_arr, "wv": wv_arr,
        })
    return in_maps


def combine_outputs(results, per_core, valid_lens):
    o_acc = np.zeros((B, 256, Q), np.float64)
    s_acc = np.zeros((B, Q), np.float64)
    for core in range(N_CORES):
        o = results[core]["o_out"]   # [T, 128, DC, Q]
        s = results[core]["s_out"]   # [T, 1, Q]
        for t, task in enumerate(per_core[core]):
            if task is None:
                continue
            b, _ = task
            # o[t][dp, dc, q] -> full d = dc*128 + dp
            o_acc[b] += o[t].transpose(1, 0, 2).reshape(256, Q)
            s_acc[b] += s[t][0]
    out = o_acc / s_acc[:, None, :]          # [B, D, Q]
    return np.ascontiguousarray(out.transpose(0, 2, 1)).astype(np.float32)


def kernel(queries, keys, values, valid_lens, W_q, W_k, w_v, _run_kwargs=None):
    queries = np.asarray(queries, np.float32)
    keys = np.asarray(keys, np.float32)
    values = np.asarray(values, np.float32)
    valid_lens = np.asarray(valid_lens)
    W_q = np.asarray(W_q, np.float32)
    W_k = np.asarray(W_k, np.float32)
    w_v = np.asarray(w_v, np.float32)

    per_core, n_tasks = make_task_list(valid_lens)
    nc, _ = build_nc(n_tasks)
    in_maps = pack_inputs(queries, keys, values, valid_lens, W_q, W_k, w_v,
                          per_core, n_tasks)
    kw = dict(_run_kwargs or {})
    res = bass_utils.run_bass_kernel_spmd(nc, in_maps, list(range(N_CORES)), **kw)
    out = combine_outputs(res.results, per_core, valid_lens)
    if _run_kwargs is not None:
        kernel._last_result = res
    return out


# revision 7
# speedup vs baseline: 1.0818x; 1.0818x over previous
"""AdditiveAttention Bass kernel for 8 Trainium2 NeuronCores.

Math (reference):
    q = queries @ W_q            [B,Q,H]
    k = keys @ W_k               [B,K,H]
    scores[b,q,k] = sum_h w_v[h] * tanh(q[b,q,h] + k[b,k,h])
    attn = softmax(mask(scores)) over K
    out = attn @ values          [B,Q,D]

Key observations exploited here:
  * Masked positions (k >= valid_len[b]) contribute exactly zero to the
    softmax (exp(-1e6 - max) underflows to 0 in fp32), so they can be
    skipped entirely.  valid_lens is host-visible inside kernel(), so the
    work list is built at (host) compile time.
  * |scores| <= ||w_v||_1 ~= 13, so softmax without max-subtraction is
    numerically safe in fp32; partial sums (o = sum exp(s)*v, z = sum
    exp(s)) are therefore linear and can be summed across chunks on host.
  * Work is packed into uniform (batch, key-chunk-of-128) tasks spread
    round-robin over the 8 cores -> a single SPMD program, perfectly
    load-balanced regardless of the valid_lens distribution.

Per-task device pipeline (C = 128 keys/task):
    PE : q_proj/k_proj projections (H on partitions)
    DVE: qk[h, q, c] = k_proj[h, c] + q_proj[h, q]   (per-partition scalar add)
    ACT: feat = tanh(qk) -> bf16, large free-dim instructions
    PE : scoresT[c, q] = feat[h,(q),c].T @ w_v       (feat as stationary)
    ACT: p = exp(scoresT)                            [c, q]
    PE : o[d, q] = V[c,d].T @ p ; z[q] = mask.T @ p  (mask via zeroed V rows)
Host combines per-batch partials: out[b] = (sum_t o_t) / (sum_t z_t).
"""

import math
from contextlib import ExitStack

import numpy as np
import ml_dtypes

import concourse.bass as bass
import concourse.mybir as mybir
import concourse.tile as tile
from concourse import bacc, bass_utils

F32 = mybir.dt.float32
BF16 = mybir.dt.bfloat16

B, Q, K, D, H = 16, 64, 1024, 256, 256
C = 128          # keys per task
GQ = 16          # queries per tanh group
N_CORES = 8
DC = D // 128    # d chunks (2)
HC = H // 128    # h chunks (2)


def emit_kernel(tc, aps, n_tasks):
    """Emit the per-core SPMD program for n_tasks uniform tasks."""
    nc = tc.nc
    ctx = tc.ctx  # ExitStack owned by caller

    keysT = aps["keysT"]        # [T, 128, DC, C] f32   (dp, dc, c)
    queriesT = aps["queriesT"]  # [T, 128, DC, Q] f32
    vals = aps["vals"]          # [T, 128, D] f32       (c, d) lhsT for o-matmul
    maskv = aps["maskv"]        # [T, 128, 1] f32
    Wq = aps["Wq"]              # [128, DC, H] f32      (dp, dc, h)
    Wk = aps["Wk"]              # [128, DC, H] f32
    wv = aps["wv"]              # [128, HC] bf16
    o_out = aps["o_out"]        # [T, 128, DC, Q] f32   (dp, dc, q)
    s_out = aps["s_out"]        # [T, 1, Q] f32

    const_pool = ctx.enter_context(tc.tile_pool(name="const", bufs=1))
    in_pool = ctx.enter_context(tc.tile_pool(name="inp", bufs=2))
    proj_pool = ctx.enter_context(tc.tile_pool(name="proj", bufs=2))
    qk_pool = ctx.enter_context(tc.tile_pool(name="qk", bufs=3))
    feat_pool = ctx.enter_context(tc.tile_pool(name="feat", bufs=4))
    p_pool = ctx.enter_context(tc.tile_pool(name="p", bufs=2))
    out_pool = ctx.enter_context(tc.tile_pool(name="outp", bufs=2))
    ps_proj = ctx.enter_context(tc.tile_pool(name="psproj", bufs=2, space="PSUM"))
    ps_sc = ctx.enter_context(tc.tile_pool(name="pssc", bufs=2, space="PSUM"))
    ps_o = ctx.enter_context(tc.tile_pool(name="pso", bufs=2, space="PSUM"))

    # Resident constants.
    Wq_sb = const_pool.tile([128, DC, H], F32, tag="wq")
    Wk_sb = const_pool.tile([128, DC, H], F32, tag="wk")
    wv_sb = const_pool.tile([128, HC], BF16, tag="wv")
    nc.sync.dma_start(Wq_sb[:], Wq[:])
    nc.sync.dma_start(Wk_sb[:], Wk[:])
    nc.sync.dma_start(wv_sb[:], wv[:])

    for t in range(n_tasks):
        # ---- input DMA ----
        k_sb = in_pool.tile([128, DC, C], F32, tag="k")
        qT_sb = in_pool.tile([128, DC, Q], F32, tag="q")
        v_sb = in_pool.tile([128, D], F32, tag="v")
        m_sb = in_pool.tile([128, 1], F32, tag="m")
        nc.sync.dma_start(k_sb[:], keysT[t])
        nc.sync.dma_start(qT_sb[:], queriesT[t])
        nc.sync.dma_start(v_sb[:], vals[t])
        nc.sync.dma_start(m_sb[:], maskv[t])

        # ---- projections (PE): proj_ps[:, hh, 0:Q] = q_proj, [, Q:Q+C] = k_proj
        proj_ps = ps_proj.tile([128, HC, Q + C], F32, tag="proj")
        for hh in range(HC):
            for dc in range(DC):
                nc.tensor.matmul(
                    proj_ps[:, hh, 0:Q],
                    lhsT=Wq_sb[:, dc, hh * 128:(hh + 1) * 128],
                    rhs=qT_sb[:, dc, :],
                    start=(dc == 0), stop=(dc == DC - 1),
                )
            for dc in range(DC):
                nc.tensor.matmul(
                    proj_ps[:, hh, Q:Q + C],
                    lhsT=Wk_sb[:, dc, hh * 128:(hh + 1) * 128],
                    rhs=k_sb[:, dc, :],
                    start=(dc == 0), stop=(dc == DC - 1),
                )
        qp_sb = proj_pool.tile([128, HC, Q], F32, tag="qp")
        kp_sb = proj_pool.tile([128, HC, C], BF16, tag="kp")
        nc.scalar.copy(qp_sb[:], proj_ps[:, :, 0:Q])
        nc.vector.tensor_copy(kp_sb[:], proj_ps[:, :, Q:Q + C])

        # ---- qk broadcast-add (DVE) + tanh (ACT), grouped by GQ queries ----
        feats = []
        for g in range(Q // GQ):
            qk = qk_pool.tile([128, GQ, HC, C], BF16, tag="qk")
            for i in range(GQ):
                qq = g * GQ + i
                for hh in range(HC):
                    nc.vector.tensor_scalar_add(
                        qk[:, i, hh, :],
                        kp_sb[:, hh, :],
                        qp_sb[:, hh, qq:qq + 1],
                    )
            feat = feat_pool.tile([128, GQ, HC, C], BF16, tag="feat")
            nc.scalar.activation(feat[:], qk[:], mybir.ActivationFunctionType.Tanh)
            feats.append(feat)

        # ---- scoresT[c, q] (PE): feat as stationary, w_v streaming ----
        sc_ps = ps_sc.tile([128, Q + Q], F32, tag="sc")  # [:, :Q] scoresT, [0:1, Q:] z
        for qq in range(Q):
            g, i = divmod(qq, GQ)
            for hh in range(HC):
                nc.tensor.matmul(
                    sc_ps[:, qq:qq + 1],
                    lhsT=feats[g][:, i, hh, :],
                    rhs=wv_sb[:, hh:hh + 1],
                    start=(hh == 0), stop=(hh == HC - 1),
                )

        # ---- exp (ACT) ----
        p_sb = p_pool.tile([128, Q], F32, tag="p")
        nc.scalar.activation(p_sb[:], sc_ps[:, 0:Q], mybir.ActivationFunctionType.Exp)

        # ---- o = V.T @ p, z = mask.T @ p (PE) ----
        o_ps = ps_o.tile([128, DC, Q], F32, tag="o")
        for dc in range(DC):
            nc.tensor.matmul(
                o_ps[:, dc, :],
                lhsT=v_sb[:, dc * 128:(dc + 1) * 128],
                rhs=p_sb[:],
                start=True, stop=True,
            )
        nc.tensor.matmul(
            sc_ps[0:1, Q:Q + Q],
            lhsT=m_sb[:],
            rhs=p_sb[:],
            start=True, stop=True,
        )

        # ---- evacuate + output DMA ----
        o_sb = out_pool.tile([128, DC, Q], F32, tag="osb")
        s_sb = out_pool.tile([1, Q], F32, tag="ssb")
        nc.scalar.copy(o_sb[:], o_ps[:])
        nc.scalar.copy(s_sb[:], sc_ps[0:1, Q:Q + Q])
        nc.sync.dma_start(o_out[t], o_sb[:])
        nc.sync.dma_start(s_out[t], s_sb[:])


_NC_CACHE = {}


def build_nc(n_tasks):
    if n_tasks in _NC_CACHE:
        return _NC_CACHE[n_tasks]
    nc = bacc.Bacc("TRN2", target_bir_lowering=False, debug=False)
    aps = {
        "keysT": nc.dram_tensor("keysT", [n_tasks, 128, DC, C], F32,
                                kind="ExternalInput").ap(),
        "queriesT": nc.dram_tensor("queriesT", [n_tasks, 128, DC, Q], F32,
                                   kind="ExternalInput").ap(),
        "vals": nc.dram_tensor("vals", [n_tasks, 128, D], F32,
                               kind="ExternalInput").ap(),
        "maskv": nc.dram_tensor("maskv", [n_tasks, 128, 1], F32,
                                kind="ExternalInput").ap(),
        "Wq": nc.dram_tensor("Wq", [128, DC, H], F32, kind="ExternalInput").ap(),
        "Wk": nc.dram_tensor("Wk", [128, DC, H], F32, kind="ExternalInput").ap(),
        "wv": nc.dram_tensor("wv", [128, HC], BF16, kind="ExternalInput").ap(),
        "o_out": nc.dram_tensor("o_out", [n_tasks, 128, DC, Q], F32,
                                kind="ExternalOutput").ap(),
        "s_out": nc.dram_tensor("s_out", [n_tasks, 1, Q], F32,
                                kind="ExternalOutput").ap(),
    }
    with tile.TileContext(nc) as tc:
        with ExitStack() as stack:
            tc.ctx = stack
            emit_kernel(tc, aps, n_tasks)
    nc.compile()
    _NC_CACHE[n_tasks] = (nc, aps)
    return nc, aps


def make_task_list(valid_lens):
    """Uniform (b, c0) tasks; None = dummy task. Returns (tasks, n_tasks)."""
    chunks = []
    for b in range(B):
        v = int(valid_lens[b])
        for c0 in range(0, v, C):
            chunks.append((b, c0))
    n_tasks = math.ceil(len(chunks) / N_CORES)
    chunks += [None] * (n_tasks * N_CORES - len(chunks))
    per_core = [chunks[i * n_tasks:(i + 1) * n_tasks] for i in range(N_CORES)]
    return per_core, n_tasks


def pack_inputs(queries, keys, values, valid_lens, W_q, W_k, w_v, per_core, n_tasks):
    """Build the per-core input maps (all little host-side layout work)."""
    Wq_arr = np.ascontiguousarray(
        W_q.reshape(DC, 128, H).transpose(1, 0, 2))       # [128, DC, H]
    Wk_arr = np.ascontiguousarray(
        W_k.reshape(DC, 128, H).transpose(1, 0, 2))
    wv_arr = np.ascontiguousarray(
        w_v.reshape(HC, 128).T.astype(ml_dtypes.bfloat16))  # [128, HC]

    in_maps = []
    for core in range(N_CORES):
        keysT = np.zeros((n_tasks, 128, DC, C), np.float32)
        queriesT = np.zeros((n_tasks, 128, DC, Q), np.float32)
        vals = np.zeros((n_tasks, 128, D), np.float32)
        maskv = np.zeros((n_tasks, 128, 1), np.float32)
        for t, task in enumerate(per_core[core]):
            if task is None:
                continue
            b, c0 = task
            v = int(valid_lens[b])
            n = min(C, v - c0)
            # keys[b, c0:c0+n, :] -> [D, n] -> [128dp, DC, n]
            kT = keys[b, c0:c0 + n, :].T.reshape(DC, 128, n)
            keysT[t, :, :, :n] = kT.transpose(1, 0, 2)
            qT = queries[b].T.reshape(DC, 128, Q)
            queriesT[t] = qT.transpose(1, 0, 2)
            vals[t, :n, :] = values[b, c0:c0 + n, :]
            maskv[t, :n, 0] = 1.0
        in_maps.append({
            "keysT": keysT, "queriesT": queriesT, "vals": vals, "maskv": maskv,
            "Wq": Wq_arr, "Wk": Wk_arr, "wv": wv_arr,
        })
    return in_maps


def combine_outputs(results, per_core, valid_lens):
    o_acc = np.zeros((B, 256, Q), np.float64)
    s_acc = np.zeros((B, Q), np.float64)
    for core in range(N_CORES):
        o = results[core]["o_out"]   # [T, 128, DC, Q]
        s = results[core]["s_out"]   # [T, 1, Q]
        for t, task in enumerate(per_core[core]):
            if task is None:
                continue
            b, _ = task
            # o[t][dp, dc, q] -> full d = dc*128 + dp
            o_acc[b] += o[t].transpose(1, 0, 2).reshape(256, Q)
            s_acc[b] += s[t][0]
    out = o_acc / s_acc[:, None, :]          # [B, D, Q]
    return np.ascontiguousarray(out.transpose(0, 2, 1)).astype(np.float32)


def kernel(queries, keys, values, valid_lens, W_q, W_k, w_v, _run_kwargs=None):
    queries = np.asarray(queries, np.float32)
    keys = np.asarray(keys, np.float32)
    values = np.asarray(values, np.float32)
    valid_lens = np.asarray(valid_lens)
    W_q = np.asarray(W_q, np.float32)
    W_k = np.asarray(W_k, np.float32)
    w_v = np.asarray(w_v, np.float32)

    per_core, n_tasks = make_task_list(valid_lens)
    nc, _ = build_nc(n_tasks)
    in_maps = pack_inputs(queries, keys, values, valid_lens, W_q, W_k, w_v,
                          per_core, n_tasks)
    kw = dict(_run_kwargs or {})
    res = bass_utils.run_bass_kernel_spmd(nc, in_maps, list(range(N_CORES)), **kw)
    out = combine_outputs(res.results, per_core, valid_lens)
    if _run_kwargs is not None:
        kernel._last_result = res
    return out


# revision 9
# speedup vs baseline: 1.2727x; 1.1765x over previous
"""AdditiveAttention Bass kernel for 8 Trainium2 NeuronCores.

Math (reference):
    q = queries @ W_q            [B,Q,H]
    k = keys @ W_k               [B,K,H]
    scores[b,q,k] = sum_h w_v[h] * tanh(q[b,q,h] + k[b,k,h])
    attn = softmax(mask(scores)) over K
    out = attn @ values          [B,Q,D]

Key observations exploited here:
  * Masked positions (k >= valid_len[b]) contribute exactly zero to the
    softmax (exp(-1e6 - max) underflows to 0 in fp32), so they can be
    skipped entirely.  valid_lens is host-visible inside kernel(), so the
    work list is built at (host) compile time.
  * |scores| <= ||w_v||_1 ~= 13, so softmax without max-subtraction is
    numerically safe in fp32; partial sums (o = sum exp(s)*v, z = sum
    exp(s)) are therefore linear and can be summed across chunks on host.
  * Work is packed into uniform (batch, key-chunk-of-128) tasks spread
    round-robin over the 8 cores -> a single SPMD program, perfectly
    load-balanced regardless of the valid_lens distribution.

Per-task device pipeline (C = 128 keys/task):
    PE : q_proj/k_proj projections (H on partitions)
    DVE: qk[h, q, c] = k_proj[h, c] + q_proj[h, q]   (per-partition scalar add)
    ACT: feat = tanh(qk) -> bf16, large free-dim instructions
    PE : scoresT[c, q] = feat[h,(q),c].T @ w_v       (feat as stationary)
    ACT: p = exp(scoresT)                            [c, q]
    PE : o[d, q] = V[c,d].T @ p ; z[q] = mask.T @ p  (mask via zeroed V rows)
Host combines per-batch partials: out[b] = (sum_t o_t) / (sum_t z_t).
"""

import math
from contextlib import ExitStack

import numpy as np
import ml_dtypes

import concourse.bass as bass
import concourse.mybir as mybir
import concourse.tile as tile
from concourse import bacc, bass_utils

F32 = mybir.dt.float32
BF16 = mybir.dt.bfloat16

B, Q, K, D, H = 16, 64, 1024, 256, 256
C = 128          # keys per task
GQ = 16          # queries per tanh group (DVE+ACT path)
N_DIRECT = 7     # queries per task routed via ACT-direct (bias-fused tanh)
N_CORES = 8
DC = D // 128    # d chunks (2)
HC = H // 128    # h chunks (2)


def emit_kernel(tc, aps, n_tasks):
    """Emit the per-core SPMD program for n_tasks uniform tasks."""
    nc = tc.nc
    ctx = tc.ctx  # ExitStack owned by caller

    keysT = aps["keysT"]        # [T, 128, DC, C] f32   (dp, dc, c)
    queriesT = aps["queriesT"]  # [T, 128, DC, Q] f32
    vals = aps["vals"]          # [T, 128, D] f32       (c, d) lhsT for o-matmul
    maskv = aps["maskv"]        # [T, 128, 1] f32
    Wq = aps["Wq"]              # [128, DC, H] f32      (dp, dc, h)
    Wk = aps["Wk"]              # [128, DC, H] f32
    wv = aps["wv"]              # [128, HC] bf16
    o_out = aps["o_out"]        # [T, 128, DC, Q] f32   (dp, dc, q)
    s_out = aps["s_out"]        # [T, 1, Q] f32

    const_pool = ctx.enter_context(tc.tile_pool(name="const", bufs=1))
    in_pool = ctx.enter_context(tc.tile_pool(name="inp", bufs=2))
    proj_pool = ctx.enter_context(tc.tile_pool(name="proj", bufs=2))
    qk_pool = ctx.enter_context(tc.tile_pool(name="qk", bufs=3))
    feat_pool = ctx.enter_context(tc.tile_pool(name="feat", bufs=4))
    p_pool = ctx.enter_context(tc.tile_pool(name="p", bufs=2))
    out_pool = ctx.enter_context(tc.tile_pool(name="outp", bufs=2))
    ps_proj = ctx.enter_context(tc.tile_pool(name="psproj", bufs=2, space="PSUM"))
    ps_sc = ctx.enter_context(tc.tile_pool(name="pssc", bufs=2, space="PSUM"))
    ps_o = ctx.enter_context(tc.tile_pool(name="pso", bufs=2, space="PSUM"))

    # Resident constants.
    Wq_sb = const_pool.tile([128, DC, H], F32, tag="wq")
    Wk_sb = const_pool.tile([128, DC, H], F32, tag="wk")
    wv_sb = const_pool.tile([128, HC], BF16, tag="wv")
    nc.sync.dma_start(Wq_sb[:], Wq[:])
    nc.sync.dma_start(Wk_sb[:], Wk[:])
    nc.sync.dma_start(wv_sb[:], wv[:])

    # q-group layout: first Q - N_DIRECT queries via DVE add + grouped tanh,
    # last N_DIRECT queries via ACT-direct (bias-fused tanh straight from kp).
    n_big = Q - N_DIRECT
    groups = []  # (q_start, q_len)
    q0 = 0
    while q0 < n_big:
        ln = min(GQ, n_big - q0)
        groups.append((q0, ln))
        q0 += ln

    def emit_inputs_and_proj(t):
        """DMA inputs + projections + evacuation for task t."""
        k_sb = in_pool.tile([128, DC, C], F32, tag="k")
        qT_sb = in_pool.tile([128, DC, Q], F32, tag="q")
        v_sb = in_pool.tile([128, D], F32, tag="v")
        m_sb = in_pool.tile([128, 1], F32, tag="m")
        nc.sync.dma_start(k_sb[:], keysT[t])
        nc.sync.dma_start(qT_sb[:], queriesT[t])
        nc.sync.dma_start(v_sb[:], vals[t])
        nc.sync.dma_start(m_sb[:], maskv[t])

        # proj_ps[:, hh, 0:Q] = q_proj, [:, hh, Q:Q+C] = k_proj
        proj_ps = ps_proj.tile([128, HC, Q + C], F32, tag="proj")
        for hh in range(HC):
            for dc in range(DC):
                nc.tensor.matmul(
                    proj_ps[:, hh, 0:Q],
                    lhsT=Wq_sb[:, dc, hh * 128:(hh + 1) * 128],
                    rhs=qT_sb[:, dc, :],
                    start=(dc == 0), stop=(dc == DC - 1),
                )
            for dc in range(DC):
                nc.tensor.matmul(
                    proj_ps[:, hh, Q:Q + C],
                    lhsT=Wk_sb[:, dc, hh * 128:(hh + 1) * 128],
                    rhs=k_sb[:, dc, :],
                    start=(dc == 0), stop=(dc == DC - 1),
                )
        qp_sb = proj_pool.tile([128, HC, Q], F32, tag="qp")
        kp_sb = proj_pool.tile([128, HC, C], BF16, tag="kp")
        nc.scalar.copy(qp_sb[:], proj_ps[:, :, 0:Q])
        nc.vector.tensor_copy(kp_sb[:], proj_ps[:, :, Q:Q + C])
        return k_sb, qT_sb, v_sb, m_sb, qp_sb, kp_sb

    state = {}

    for t in range(n_tasks):
        if t == 0:
            state[0] = emit_inputs_and_proj(0)
        _, _, v_sb, m_sb, qp_sb, kp_sb = state.pop(t)
        if t + 1 < n_tasks:
            # Pipelined: next task's projections go ahead of this task's
            # scores in the PE stream, so PE/DVE never stall on task turnover.
            state[t + 1] = emit_inputs_and_proj(t + 1)

        # ---- qk broadcast-add (DVE) + tanh (ACT), grouped queries ----
        feats = []   # (tile, local_idx) per big-path query
        for (q0, ln) in groups:
            qk = qk_pool.tile([128, GQ, HC, C], BF16, tag="qk")
            for i in range(ln):
                qq = q0 + i
                for hh in range(HC):
                    nc.vector.tensor_scalar_add(
                        qk[:, i, hh, :],
                        kp_sb[:, hh, :],
                        qp_sb[:, hh, qq:qq + 1],
                    )
            feat = feat_pool.tile([128, GQ, HC, C], BF16, tag="feat")
            nc.scalar.activation(feat[:, 0:ln], qk[:, 0:ln],
                                 mybir.ActivationFunctionType.Tanh)
            for i in range(ln):
                feats.append((feat, i))
        # ACT-direct queries: tanh(kp + qp[q]) with per-partition bias
        for qq in range(n_big, Q):
            featd = feat_pool.tile([128, 1, HC, C], BF16, tag="featd")
            for hh in range(HC):
                nc.scalar.activation(
                    featd[:, 0, hh, :], kp_sb[:, hh, :],
                    mybir.ActivationFunctionType.Tanh,
                    bias=qp_sb[:, hh, qq:qq + 1],
                )
            feats.append((featd, 0))

        # ---- scoresT[c, q] (PE): feat as stationary, w_v streaming ----
        sc_ps = ps_sc.tile([128, Q + Q], F32, tag="sc")  # [:, :Q] scoresT, [0:1, Q:] z
        for qq in range(Q):
            ftile, i = feats[qq]
            for hh in range(HC):
                nc.tensor.matmul(
                    sc_ps[:, qq:qq + 1],
                    lhsT=ftile[:, i, hh, :],
                    rhs=wv_sb[:, hh:hh + 1],
                    start=(hh == 0), stop=(hh == HC - 1),
                )

        # ---- exp (ACT) ----
        p_sb = p_pool.tile([128, Q], F32, tag="p")
        nc.scalar.activation(p_sb[:], sc_ps[:, 0:Q], mybir.ActivationFunctionType.Exp)

        # ---- o = V.T @ p, z = mask.T @ p (PE) ----
        o_ps = ps_o.tile([128, DC, Q], F32, tag="o")
        for dc in range(DC):
            nc.tensor.matmul(
                o_ps[:, dc, :],
                lhsT=v_sb[:, dc * 128:(dc + 1) * 128],
                rhs=p_sb[:],
                start=True, stop=True,
            )
        nc.tensor.matmul(
            sc_ps[0:1, Q:Q + Q],
            lhsT=m_sb[:],
            rhs=p_sb[:],
            start=True, stop=True,
        )

        # ---- evacuate + output DMA ----
        o_sb = out_pool.tile([128, DC, Q], F32, tag="osb")
        s_sb = out_pool.tile([1, Q], F32, tag="ssb")
        nc.scalar.copy(o_sb[:], o_ps[:])
        nc.scalar.copy(s_sb[:], sc_ps[0:1, Q:Q + Q])
        nc.sync.dma_start(o_out[t], o_sb[:])
        nc.sync.dma_start(s_out[t], s_sb[:])


_NC_CACHE = {}


def build_nc(n_tasks):
    if n_tasks in _NC_CACHE:
        return _NC_CACHE[n_tasks]
    nc = bacc.Bacc("TRN2", target_bir_lowering=False, debug=False)
    aps = {
        "keysT": nc.dram_tensor("keysT", [n_tasks, 128, DC, C], F32,
                                kind="ExternalInput").ap(),
        "queriesT": nc.dram_tensor("queriesT", [n_tasks, 128, DC, Q], F32,
                                   kind="ExternalInput").ap(),
        "vals": nc.dram_tensor("vals", [n_tasks, 128, D], F32,
                               kind="ExternalInput").ap(),
        "maskv": nc.dram_tensor("maskv", [n_tasks, 128, 1], F32,
                                kind="ExternalInput").ap(),
        "Wq": nc.dram_tensor("Wq", [128, DC, H], F32, kind="ExternalInput").ap(),
        "Wk": nc.dram_tensor("Wk", [128, DC, H], F32, kind="ExternalInput").ap(),
        "wv": nc.dram_tensor("wv", [128, HC], BF16, kind="ExternalInput").ap(),
        "o_out": nc.dram_tensor("o_out", [n_tasks, 128, DC, Q], F32,
                                kind="ExternalOutput").ap(),
        "s_out": nc.dram_tensor("s_out", [n_tasks, 1, Q], F32,
                                kind="ExternalOutput").ap(),
    }
    with tile.TileContext(nc) as tc:
        with ExitStack() as stack:
            tc.ctx = stack
            emit_kernel(tc, aps, n_tasks)
    nc.compile()
    _NC_CACHE[n_tasks] = (nc, aps)
    return nc, aps


def make_task_list(valid_lens):
    """Uniform (b, c0) tasks; None = dummy task. Returns (tasks, n_tasks)."""
    chunks = []
    for b in range(B):
        v = int(valid_lens[b])
        for c0 in range(0, v, C):
            chunks.append((b, c0))
    n_tasks = math.ceil(len(chunks) / N_CORES)
    chunks += [None] * (n_tasks * N_CORES - len(chunks))
    per_core = [chunks[i * n_tasks:(i + 1) * n_tasks] for i in range(N_CORES)]
    return per_core, n_tasks


def pack_inputs(queries, keys, values, valid_lens, W_q, W_k, w_v, per_core, n_tasks):
    """Build the per-core input maps (all little host-side layout work)."""
    Wq_arr = np.ascontiguousarray(
        W_q.reshape(DC, 128, H).transpose(1, 0, 2))       # [128, DC, H]
    Wk_arr = np.ascontiguousarray(
        W_k.reshape(DC, 128, H).transpose(1, 0, 2))
    wv_arr = np.ascontiguousarray(
        w_v.reshape(HC, 128).T.astype(ml_dtypes.bfloat16))  # [128, HC]

    in_maps = []
    for core in range(N_CORES):
        keysT = np.zeros((n_tasks, 128, DC, C), np.float32)
        queriesT = np.zeros((n_tasks, 128, DC, Q), np.float32)
        vals = np.zeros((n_tasks, 128, D), np.float32)
        maskv = np.zeros((n_tasks, 128, 1), np.float32)
        for t, task in enumerate(per_core[core]):
            if task is None:
                continue
            b, c0 = task
            v = int(valid_lens[b])
            n = min(C, v - c0)
            # keys[b, c0:c0+n, :] -> [D, n] -> [128dp, DC, n]
            kT = keys[b, c0:c0 + n, :].T.reshape(DC, 128, n)
            keysT[t, :, :, :n] = kT.transpose(1, 0, 2)
            qT = queries[b].T.reshape(DC, 128, Q)
            queriesT[t] = qT.transpose(1, 0, 2)
            vals[t, :n, :] = values[b, c0:c0 + n, :]
            maskv[t, :n, 0] = 1.0
        in_maps.append({
            "keysT": keysT, "queriesT": queriesT, "vals": vals, "maskv": maskv,
            "Wq": Wq_arr, "Wk": Wk_arr, "wv": wv_arr,
        })
    return in_maps


def combine_outputs(results, per_core, valid_lens):
    o_acc = np.zeros((B, 256, Q), np.float64)
    s_acc = np.zeros((B, Q), np.float64)
    for core in range(N_CORES):
        o = results[core]["o_out"]   # [T, 128, DC, Q]
        s = results[core]["s_out"]   # [T, 1, Q]
        for t, task in enumerate(per_core[core]):
            if task is None:
                continue
            b, _ = task
            # o[t][dp, dc, q] -> full d = dc*128 + dp
            o_acc[b] += o[t].transpose(1, 0, 2).reshape(256, Q)
            s_acc[b] += s[t][0]
    out = o_acc / s_acc[:, None, :]          # [B, D, Q]
    return np.ascontiguousarray(out.transpose(0, 2, 1)).astype(np.float32)


def kernel(queries, keys, values, valid_lens, W_q, W_k, w_v, _run_kwargs=None):
    queries = np.asarray(queries, np.float32)
    keys = np.asarray(keys, np.float32)
    values = np.asarray(values, np.float32)
    valid_lens = np.asarray(valid_lens)
    W_q = np.asarray(W_q, np.float32)
    W_k = np.asarray(W_k, np.float32)
    w_v = np.asarray(w_v, np.float32)

    per_core, n_tasks = make_task_list(valid_lens)
    nc, _ = build_nc(n_tasks)
    in_maps = pack_inputs(queries, keys, values, valid_lens, W_q, W_k, w_v,
                          per_core, n_tasks)
    kw = dict(_run_kwargs or {})
    res = bass_utils.run_bass_kernel_spmd(nc, in_maps, list(range(N_CORES)), **kw)
    out = combine_outputs(res.results, per_core, valid_lens)
    if _run_kwargs is not None:
        kernel._last_result = res
    return out
